# revision 10
# baseline (speedup 1.0000x reference)
"""Multi-head attention (B=4, T=2048, D=1024, H=16, causal) on 8 trn2 cores.

Sharding: core c handles batch b=c//2 and head-group hg=c%2 (8 global heads),
processed as 2 passes of 4 heads. Host sums the two head-group partials per
batch (out-projection is linear in heads) and adds b_out.

Per-core kernel (per pass of 4 local heads):
  1. xT via PE transposes; qT,kT computed feature-major [256+256, T];
     v computed token-major [T, 4*65] with a ones column per head
     (the ones column makes the PV matmul emit the softmax denominator).
  2. scoresT[k,q] = kT.T @ qT per (head, 128k x 512q) block, causal blocks
     skipped, partial blocks get an additive -1e9 mask; exp on ACT with the
     1/sqrt(hd) scale folded in (no max-subtraction: scores ~ N(0,1)).
  3. outT[65,512] = v~.T @ p accumulated over k-tiles; row 64 = denominator.
     Unnormalized rows are copied into yT (partition-shifted for odd heads),
     denominators gathered into one [16,512] tile -> one batched reciprocal
     -> two-hot selector matmul broadcasts 1/den per head pair -> one DVE
     multiply normalizes yT in place.
  4. out-proj from yT (head pairs packed, K=128) -> partial output.

Matmuls run in float32r (fp32 bits, single-pass PE streaming, tf32-class
precision ~2e-4 rel). Set KERNEL_MM_DT=f32 for exact-fp32 (2-pass, slower).
"""

import os
import sys

sys.path.insert(0, "/opt/trn_rl_repo")

import numpy as np

from concourse import bacc, mybir, tile
from concourse.tile import add_dep_helper
from concourse.bass_utils import run_bass_kernel_spmd

f32 = mybir.dt.float32
MMDT = f32 if os.environ.get("KERNEL_MM_DT") == "f32" else mybir.dt.float32r
AF = mybir.ActivationFunctionType

B, T, D, H = 4, 2048, 1024, 16
HD = D // H                     # 64
NH = 4                          # local heads per pass
NPASS = 2                       # head passes per core
F = NH * HD                     # 256 features per pass for q, k and v
NKT = T // 128                  # 16 k tiles
NQJ = T // 512                  # 4 q column blocks
NCH = 4                         # token chunks for projection
CH = T // NCH                   # 512 tokens per chunk

NEG = -1.0e9

_CACHE = {}
LAST_RESULTS = None


def _classify_blocks(mask):
    """mask: [T, T] bool, mask[q, k]. Returns (blocks, patterns) where
    blocks[(ki, qj)] in {"full", "skip", int u} and patterns is [U, 128, 512]
    additive f32 masks in scoresT layout [k, q]."""
    blocks = {}
    patterns = []
    seen = {}
    for ki in range(NKT):
        for qj in range(NQJ):
            sub = mask[qj * 512:(qj + 1) * 512, ki * 128:(ki + 1) * 128]
            if sub.all():
                blocks[(ki, qj)] = "full"
            elif not sub.any():
                blocks[(ki, qj)] = "skip"
            else:
                pat = np.where(sub.T, 0.0, NEG).astype(np.float32)  # [128k, 512q]
                key = pat.tobytes()
                if key not in seen:
                    seen[key] = len(patterns)
                    patterns.append(pat)
                blocks[(ki, qj)] = seen[key]
    if not patterns:
        patterns.append(np.zeros((128, 512), np.float32))
    return blocks, np.stack(patterns)


def _build(blocks, n_pat):
    nc = bacc.Bacc(None)

    x_d = nc.declare_dram_parameter("xb", [T, D], MMDT, isOutput=False)
    wqk_d = nc.declare_dram_parameter("wqk", [D, NPASS, 2 * F], MMDT, isOutput=False)
    bqk_d = nc.declare_dram_parameter("bqk", [NPASS, 2 * F, 1], f32, isOutput=False)
    wv_d = nc.declare_dram_parameter("wv", [D, NPASS, NH * 65], MMDT, isOutput=False)
    bv_d = nc.declare_dram_parameter("bv", [NPASS, 1, NH * 65], MMDT, isOutput=False)
    wo_d = nc.declare_dram_parameter("wo", [NPASS, NH // 2, 128, D], MMDT,
                                     isOutput=False)
    pm_d = nc.declare_dram_parameter("pm", [n_pat, 128, 512], f32, isOutput=False)
    id_d = nc.declare_dram_parameter("ident", [128, 128], MMDT, isOutput=False)
    sel_d = nc.declare_dram_parameter("sel", [8, 16, 128], f32, isOutput=False)
    ones_d = nc.declare_dram_parameter("onesd", [1, 128], MMDT, isOutput=False)
    out_d = nc.declare_dram_parameter("out", [NPASS, T, D], f32, isOutput=True)

    with tile.TileContext(nc) as tc:
        with tc.tile_pool(name="const", bufs=1) as cpool:
            id_sb = cpool.tile([128, 128], MMDT, name="id_sb")
            nc.sync.dma_start(id_sb[:], id_d[:])
            pm_sb = []
            for u in range(n_pat):
                t_ = cpool.tile([128, 512], f32, name=f"pm{u}", tag=f"pm{u}")
                nc.sync.dma_start(t_[:], pm_d[u])
                pm_sb.append(t_)
            sel_sb = []
            for i in range(8):
                t_ = cpool.tile([16, 128], f32, name=f"sel{i}", tag=f"sel{i}")
                nc.sync.dma_start(t_[:], sel_d[i])
                sel_sb.append(t_)
            ones_tok = cpool.tile([1, 128], MMDT, name="ones_tok")
            nc.sync.dma_start(ones_tok[:], ones_d[:])
            hot_sb = cpool.tile([1, 8], mybir.dt.bfloat16, name="hot_sb")
            nc.vector.memset(hot_sb[:], 1.0)

            with (
                tc.tile_pool(name="xtpers", bufs=1) as xtpers,
                tc.tile_pool(name="hot_ps", bufs=1, space="PSUM") as hot_ps,
            ):
                hot_ps_t = hot_ps.tile([1, 8], f32, name="hot_ps_t")

                def keep_warm(anchor=None):
                    mm = nc.tensor.matmul(hot_ps_t[:], hot_sb[0:1, 0:1],
                                          hot_sb[0:1, :], start=True, stop=True)
                    if anchor is not None:
                        add_dep_helper(mm.ins, anchor.ins, False, "ham keepwarm")
                    return mm

                xT = [xtpers.tile([128, T], MMDT, name=f"xTf{d_}", tag=f"xTf{d_}")
                      for d_ in range(8)]
                with (
                    tc.tile_pool(name="xpool", bufs=4) as xpool,
                    tc.tile_pool(name="tr_ps", bufs=2, space="PSUM") as tr_ps,
                ):
                    for ch in range(NCH):
                        x_sb = []
                        for tt in range(4):
                            r0 = ch * CH + tt * 128
                            xt_ = xpool.tile([128, D], MMDT, name="x_sb", tag="x")
                            nc.sync.dma_start(xt_[:], x_d[r0:r0 + 128, :])
                            x_sb.append(xt_)
                        for d_ in range(8):
                            for tt in range(4):
                                tp = tr_ps.tile([128, 128], MMDT, name="tp", tag="tp")
                                tr = nc.tensor.transpose(
                                    tp[:], x_sb[tt][:, d_ * 128:(d_ + 1) * 128],
                                    id_sb[:])
                                nc.vector.tensor_copy(
                                    xT[d_][:, ch * CH + tt * 128:
                                           ch * CH + (tt + 1) * 128], tp[:])
                                if (d_ * 4 + tt) % 4 == 0:
                                    keep_warm(tr)
                for p in range(NPASS):
                    _emit_pass(nc, tc, p, blocks, pm_sb, sel_sb, ones_tok, xT,
                               keep_warm, x_d, wqk_d, bqk_d, wv_d, bv_d, wo_d,
                               out_d)

    nc.compile()
    return nc


def _emit_pass(nc, tc, p, blocks, pm_sb, sel_sb, ones_tok, xT, keep_warm,
               x_d, wqk_d, bqk_d, wv_d, bv_d, wo_d, out_d):
    with (
        tc.tile_pool(name=f"persist{p}", bufs=1) as pers,
        tc.tile_pool(name=f"wpool{p}", bufs=1) as wpool,
    ):
        # persistent per-pass tensors
        qkT = [pers.tile([128, T], MMDT, name=f"qkT{p}_{m}", tag=f"qkT{m}")
               for m in range(4)]                       # m 0,1 = q; 2,3 = k
        vA = [pers.tile([128, NH * 65], MMDT, name=f"vA{p}_{i}", tag=f"vA{i}")
              for i in range(NKT)]                      # [tok, (h, hd+1)]
        yT2 = [pers.tile([128, T], MMDT, name=f"yT2{p}_{hp}", tag=f"yT2{hp}")
               for hp in range(NH // 2)]
        dgather = pers.tile([16, 512], f32, name=f"dg{p}", tag="dg")
        rgather = pers.tile([16, 512], f32, name=f"rg{p}", tag="rg")

        # weights
        wqk_sb = [wpool.tile([128, 2 * F], MMDT, name=f"wqk{p}_{k}", tag=f"wqk{k}")
                  for k in range(8)]
        wv_sb = [wpool.tile([128, NH * 65], MMDT, name=f"wv{p}_{k}", tag=f"wv{k}")
                 for k in range(8)]
        bqk_sb = [wpool.tile([128, 1], f32, name=f"bqk{p}_{m}", tag=f"bqk{m}")
                  for m in range(4)]
        bv_sb = wpool.tile([1, NH * 65], MMDT, name=f"bv{p}", tag="bv")
        wo_sb = [wpool.tile([128, D], MMDT, name=f"wo{p}_{hp}", tag=f"wo{hp}")
                 for hp in range(NH // 2)]
        for k in range(8):
            nc.sync.dma_start(wqk_sb[k][:], wqk_d[k * 128:(k + 1) * 128, p, :])
            nc.sync.dma_start(wv_sb[k][:], wv_d[k * 128:(k + 1) * 128, p, :])
        for m in range(4):
            nc.sync.dma_start(bqk_sb[m][:], bqk_d[p, m * 128:(m + 1) * 128, :])
        nc.sync.dma_start(bv_sb[:], bv_d[p])
        for hp in range(NH // 2):
            nc.sync.dma_start(wo_sb[hp][:], wo_d[p, hp])

        # ---- projection (consumes shared xT) ----
        with (
            tc.tile_pool(name=f"pj_ps{p}", bufs=2, space="PSUM") as pj_ps,
        ):
            for ch in range(NCH):
                c0_, c1_ = ch * CH, (ch + 1) * CH
                # q,k projection: feature-major
                for m in range(4):
                    ps = pj_ps.tile([128, CH], f32, name="qk_ps", tag="qk_ps")
                    for k in range(8):
                        mm = nc.tensor.matmul(
                            ps[:], wqk_sb[k][:, m * 128:(m + 1) * 128],
                            xT[k][:, c0_:c1_], start=(k == 0), stop=(k == 7))
                        if k == 3:
                            keep_warm(mm)
                    nc.vector.tensor_scalar_add(
                        qkT[m][:, c0_:c1_], ps[:], bqk_sb[m][:])
                # v projection: token-major, with bias via K=1 matmul
                for tt in range(4):
                    ps = pj_ps.tile([128, NH * 65], f32, name="v_ps", tag="v_ps")
                    for k in range(8):
                        nc.tensor.matmul(
                            ps[:], xT[k][:, ch * CH + tt * 128:
                                         ch * CH + (tt + 1) * 128], wv_sb[k][:],
                            start=(k == 0), stop=False)
                    mm = nc.tensor.matmul(ps[:], ones_tok[:], bv_sb[:],
                                          start=False, stop=True)
                    keep_warm(mm)
                    ti = ch * 4 + tt
                    nc.vector.tensor_copy(vA[ti][:], ps[:])

        # ---- attention ----
        with (
            tc.tile_pool(name=f"sc_ps{p}", bufs=3, space="PSUM") as sc_ps,
            tc.tile_pool(name=f"pv_ps{p}", bufs=2, space="PSUM") as pv_ps,
            tc.tile_pool(name=f"bc_ps{p}", bufs=2, space="PSUM") as bc_ps,
            tc.tile_pool(name=f"att_sb{p}", bufs=4) as att_sb,
            tc.tile_pool(name=f"dt_sb{p}", bufs=4) as dt_sb,
        ):
            for hp in range(NH // 2):            # head pairs share row groups
                for qj in range(NQJ):
                    kis = [ki for ki in range(NKT) if blocks[(ki, qj)] != "skip"]
                    for hh in range(2):
                        h = hp * 2 + hh
                        r0 = hh * 64
                        qt, kt = qkT[hp], qkT[2 + hp]
                        pv = pv_ps.tile([65, 512], f32, name="pv", tag="pv")
                        for i, ki in enumerate(kis):
                            sc = sc_ps.tile([128, 512], f32, name="sc", tag="sc")
                            nc.tensor.matmul(
                                sc[:],
                                kt[r0:r0 + 64, ki * 128:(ki + 1) * 128],
                                qt[r0:r0 + 64, qj * 512:(qj + 1) * 512],
                                start=True, stop=True)
                            u = blocks[(ki, qj)]
                            if u != "full":
                                nc.vector.tensor_add(sc[:], sc[:], pm_sb[u][:])
                            pt = att_sb.tile([128, 512], MMDT, name="pt", tag="pt")
                            nc.scalar.activation(pt[:], sc[:], AF.Exp, scale=0.125)
                            va = vA[ki].rearrange("p (h w) -> p h w", w=65)
                            mm = nc.tensor.matmul(
                                pv[:], va[:, h, :], pt[:],
                                start=(i == 0), stop=(i == len(kis) - 1))
                            if i % 3 == 0:
                                keep_warm(mm)
                        # unnormalized y rows -> yT2 (partition-shifted for odd
                        # heads); denominator row -> dgather row h*4+qj
                        nc.scalar.copy(
                            yT2[hp][r0:r0 + 64, qj * 512:(qj + 1) * 512],
                            pv[0:64, :])
                        r = h * NQJ + qj
                        dtmp = dt_sb.tile([1, 512], f32, name="dtmp", tag="dtmp")
                        nc.vector.tensor_copy(dtmp[:], pv[64:65, :])
                        nc.sync.dma_start(dgather[r:r + 1, :], dtmp[:])
            # batched reciprocal + per-pair broadcast + in-place normalize
            nc.vector.reciprocal(rgather[:], dgather[:])
            for hp in range(NH // 2):
                for qj in range(NQJ):
                    bc = bc_ps.tile([128, 512], f32, name="bc", tag="bc")
                    nc.tensor.matmul(bc[:], sel_sb[hp * NQJ + qj][:], rgather[:],
                                     start=True, stop=True)
                    ysl = yT2[hp][:, qj * 512:(qj + 1) * 512]
                    nc.vector.tensor_mul(ysl, ysl, bc[:])

        # ---- out-projection ----
        with (
            tc.tile_pool(name=f"o_ps{p}", bufs=2, space="PSUM") as o_ps,
            tc.tile_pool(name=f"o_sb{p}", bufs=4) as o_sb,
        ):
            for tt in range(NKT):
                for n in range(2):
                    ps = o_ps.tile([128, 512], f32, name="o_ps", tag="o_ps")
                    for hp in range(NH // 2):
                        mm = nc.tensor.matmul(
                            ps[:],
                            yT2[hp][:, tt * 128:(tt + 1) * 128],
                            wo_sb[hp][:, n * 512:(n + 1) * 512],
                            start=(hp == 0), stop=(hp == NH // 2 - 1))
                    keep_warm(mm)
                    ob = o_sb.tile([128, 512], f32, name="ob", tag="ob")
                    nc.vector.tensor_copy(ob[:], ps[:])
                    nc.sync.dma_start(
                        out_d[p, tt * 128:(tt + 1) * 128, n * 512:(n + 1) * 512],
                        ob[:])


def kernel(x, mask, w_qkv, b_qkv, w_out, b_out):
    global LAST_RESULTS
    x = np.ascontiguousarray(np.asarray(x, np.float32))
    mask2d = np.asarray(mask, bool).reshape(T, T)
    w_qkv = np.asarray(w_qkv, np.float32)
    b_qkv = np.asarray(b_qkv, np.float32)
    w_out = np.asarray(w_out, np.float32)
    b_out = np.asarray(b_out, np.float32)

    blocks, patterns = _classify_blocks(mask2d)
    key = (MMDT, patterns.tobytes(), tuple(sorted(blocks.items())).__hash__())
    if key not in _CACHE:
        _CACHE[key] = _build(blocks, len(patterns))
    nc = _CACHE[key]

    ident = np.eye(128, dtype=np.float32)
    sel = np.zeros((8, 16, 128), np.float32)
    for hp in range(2):
        for qj in range(4):
            sel[hp * 4 + qj, (2 * hp) * 4 + qj, 0:64] = 1.0
            sel[hp * 4 + qj, (2 * hp + 1) * 4 + qj, 64:128] = 1.0

    in_maps = []
    for c in range(8):
        b, hg = c // 2, c % 2
        # global head range for this core: hg*8 .. hg*8+8, in 2 passes of 4
        wqk = np.empty((D, NPASS, 2 * F), np.float32)
        bqk = np.empty((NPASS, 2 * F, 1), np.float32)
        wv = np.zeros((D, NPASS, NH * 65), np.float32)
        bv = np.zeros((NPASS, 1, NH * 65), np.float32)
        wo = np.empty((NPASS, NH // 2, 128, D), np.float32)
        for p in range(NPASS):
            h0 = hg * 8 + p * NH          # first global head of this pass
            c0 = h0 * HD                  # feature offset
            wqk[:, p, 0:F] = w_qkv[:, c0:c0 + F]
            wqk[:, p, F:2 * F] = w_qkv[:, D + c0:D + c0 + F]
            bqk[p, 0:F, 0] = b_qkv[c0:c0 + F]
            bqk[p, F:2 * F, 0] = b_qkv[D + c0:D + c0 + F]
            for h in range(NH):
                cs = 2 * D + c0 + h * HD
                wv[:, p, h * 65:h * 65 + 64] = w_qkv[:, cs:cs + HD]
                bv[p, 0, h * 65:h * 65 + 64] = b_qkv[cs:cs + HD]
                bv[p, 0, h * 65 + 64] = 1.0
            for hp in range(NH // 2):
                wo[p, hp] = w_out[c0 + hp * 128:c0 + (hp + 1) * 128, :]
        in_maps.append({
            "xb": np.ascontiguousarray(x[b]),
            "wqk": wqk, "bqk": bqk, "wv": wv, "bv": bv, "wo": wo,
            "pm": patterns, "ident": ident, "sel": sel,
            "onesd": np.ones((1, 128), np.float32),
        })

    trace = os.environ.get("KERNEL_TRACE") == "1"
    LAST_RESULTS = run_bass_kernel_spmd(
        nc, in_maps, list(range(8)), trace=trace)
    res = LAST_RESULTS.results

    out = np.empty((B, T, D), np.float32)
    for b in range(B):
        acc = res[2 * b]["out"][0] + res[2 * b]["out"][1] \
            + res[2 * b + 1]["out"][0] + res[2 * b + 1]["out"][1]
        out[b] = acc + b_out
    return out


# revision 11
# speedup vs baseline: 1.8634x; 1.8634x over previous
"""Multi-head attention (B=4, T=2048, D=1024, H=16, causal) on 8 trn2 cores.

Sharding: core c handles batch b=c//2 and head-group hg=c%2 (8 global heads),
processed as 2 passes of 4 heads. Host sums the two head-group partials per
batch (out-projection is linear in heads) and adds b_out.

Per-core kernel (per pass of 4 local heads):
  1. xT via PE transposes; qT,kT computed feature-major [256+256, T];
     v computed token-major [T, 4*65] with a ones column per head
     (the ones column makes the PV matmul emit the softmax denominator).
  2. scoresT[k,q] = kT.T @ qT per (head, 128k x 512q) block, causal blocks
     skipped, partial blocks get an additive -1e9 mask; exp on ACT with the
     1/sqrt(hd) scale folded in (no max-subtraction: scores ~ N(0,1)).
  3. outT[65,512] = v~.T @ p accumulated over k-tiles; row 64 = denominator.
     Unnormalized rows are copied into yT (partition-shifted for odd heads),
     denominators gathered into one [16,512] tile -> one batched reciprocal
     -> two-hot selector matmul broadcasts 1/den per head pair -> one DVE
     multiply normalizes yT in place.
  4. out-proj from yT (head pairs packed, K=128) -> partial output.

Matmuls run in float32r (fp32 bits, single-pass PE streaming, tf32-class
precision ~2e-4 rel). Set KERNEL_MM_DT=f32 for exact-fp32 (2-pass, slower).
"""

import os
import sys

sys.path.insert(0, "/opt/trn_rl_repo")

import numpy as np

from concourse import bacc, mybir, tile
from concourse.tile import add_dep_helper
from concourse.bass_utils import run_bass_kernel_spmd

f32 = mybir.dt.float32
MMDT = f32 if os.environ.get("KERNEL_MM_DT") == "f32" else mybir.dt.float32r
AF = mybir.ActivationFunctionType

B, T, D, H = 4, 2048, 1024, 16
HD = D // H                     # 64
NH = 4                          # local heads per pass
NPASS = 2                       # head passes per core
F = NH * HD                     # 256 features per pass for q, k and v
NKT = T // 128                  # 16 k tiles
NQJ = T // 512                  # 4 q column blocks
NCH = 4                         # token chunks for projection
CH = T // NCH                   # 512 tokens per chunk

NEG = -1.0e9

_CACHE = {}
LAST_RESULTS = None


def _classify_blocks(mask):
    """mask: [T, T] bool, mask[q, k]. Returns (blocks, patterns) where
    blocks[(ki, qj)] in {"full", "skip", int u} and patterns is [U, 128, 512]
    additive f32 masks in scoresT layout [k, q]."""
    blocks = {}
    patterns = []
    seen = {}
    for ki in range(NKT):
        for qj in range(NQJ):
            sub = mask[qj * 512:(qj + 1) * 512, ki * 128:(ki + 1) * 128]
            if sub.all():
                blocks[(ki, qj)] = "full"
            elif not sub.any():
                blocks[(ki, qj)] = "skip"
            else:
                pat = np.where(sub.T, 0.0, NEG).astype(np.float32)  # [128k, 512q]
                colmasked = ~sub.any(axis=1)          # [512] col fully masked
                colany = ~sub.all(axis=1)             # [512] col has any masked
                o = 0
                while o < 512 and colmasked[o]:
                    o += 1
                anyc = np.nonzero(colany[o:])[0]
                w0 = o + int(anyc[0]) if len(anyc) else o
                w1 = o + int(anyc[-1]) + 1 if len(anyc) else o
                key = pat.tobytes()
                if key not in seen:
                    seen[key] = len(patterns)
                    patterns.append(pat)
                blocks[(ki, qj)] = (seen[key], o, w0, w1)
    if not patterns:
        patterns.append(np.zeros((128, 512), np.float32))
    return blocks, np.stack(patterns)


def _build(blocks, n_pat):
    nc = bacc.Bacc(None)

    x_d = nc.declare_dram_parameter("xb", [T, D], MMDT, isOutput=False)
    wqk_d = nc.declare_dram_parameter("wqk", [D, NPASS, 2 * F], MMDT, isOutput=False)
    bqk_d = nc.declare_dram_parameter("bqk", [NPASS, 2 * F, 1], f32, isOutput=False)
    wv_d = nc.declare_dram_parameter("wv", [D, NPASS, NH * 65], MMDT, isOutput=False)
    bv_d = nc.declare_dram_parameter("bv", [NPASS, 1, NH * 65], MMDT, isOutput=False)
    wo_d = nc.declare_dram_parameter("wo", [NPASS, NH // 2, 128, D], MMDT,
                                     isOutput=False)
    pm_d = nc.declare_dram_parameter("pm", [n_pat, 128, 512], f32, isOutput=False)
    id_d = nc.declare_dram_parameter("ident", [128, 128], MMDT, isOutput=False)
    sel_d = nc.declare_dram_parameter("sel", [8, 16, 128], f32, isOutput=False)
    ones_d = nc.declare_dram_parameter("onesd", [1, 128], MMDT, isOutput=False)
    out_d = nc.declare_dram_parameter("out", [NPASS, T, D], f32, isOutput=True)

    with tile.TileContext(nc) as tc:
        with tc.tile_pool(name="const", bufs=1) as cpool:
            id_sb = cpool.tile([128, 128], MMDT, name="id_sb")
            nc.sync.dma_start(id_sb[:], id_d[:])
            pm_sb = []
            for u in range(n_pat):
                t_ = cpool.tile([128, 512], f32, name=f"pm{u}", tag=f"pm{u}")
                nc.sync.dma_start(t_[:], pm_d[u])
                pm_sb.append(t_)
            sel_sb = []
            for i in range(8):
                t_ = cpool.tile([16, 128], f32, name=f"sel{i}", tag=f"sel{i}")
                nc.sync.dma_start(t_[:], sel_d[i])
                sel_sb.append(t_)
            ones_tok = cpool.tile([1, 128], MMDT, name="ones_tok")
            nc.sync.dma_start(ones_tok[:], ones_d[:])
            hot_sb = cpool.tile([1, 8], mybir.dt.bfloat16, name="hot_sb")
            nc.vector.memset(hot_sb[:], 1.0)

            with (
                tc.tile_pool(name="xtpers", bufs=1) as xtpers,
                tc.tile_pool(name="hot_ps", bufs=1, space="PSUM") as hot_ps,
            ):
                hot_ps_t = hot_ps.tile([1, 8], f32, name="hot_ps_t")

                def keep_warm(anchor=None):
                    mm = nc.tensor.matmul(hot_ps_t[:], hot_sb[0:1, 0:1],
                                          hot_sb[0:1, :], start=True, stop=True)
                    if anchor is not None:
                        add_dep_helper(mm.ins, anchor.ins, False, "ham keepwarm")
                    return mm

                xT = [xtpers.tile([128, T], MMDT, name=f"xTf{d_}", tag=f"xTf{d_}")
                      for d_ in range(8)]
                with (
                    tc.tile_pool(name="xpool", bufs=4) as xpool,
                    tc.tile_pool(name="tr_ps", bufs=2, space="PSUM") as tr_ps,
                ):
                    for ch in range(NCH):
                        x_sb = []
                        for tt in range(4):
                            r0 = ch * CH + tt * 128
                            xt_ = xpool.tile([128, D], MMDT, name="x_sb", tag="x")
                            nc.sync.dma_start(xt_[:], x_d[r0:r0 + 128, :])
                            x_sb.append(xt_)
                        for d_ in range(8):
                            for tt in range(4):
                                tp = tr_ps.tile([128, 128], MMDT, name="tp", tag="tp")
                                tr = nc.tensor.transpose(
                                    tp[:], x_sb[tt][:, d_ * 128:(d_ + 1) * 128],
                                    id_sb[:])
                                nc.vector.tensor_copy(
                                    xT[d_][:, ch * CH + tt * 128:
                                           ch * CH + (tt + 1) * 128], tp[:])

                for p in range(NPASS):
                    _emit_pass(nc, tc, p, blocks, pm_sb, sel_sb, ones_tok, xT,
                               keep_warm, x_d, wqk_d, bqk_d, wv_d, bv_d, wo_d,
                               out_d)

    nc.compile()
    return nc


def _emit_pass(nc, tc, p, blocks, pm_sb, sel_sb, ones_tok, xT, keep_warm,
               x_d, wqk_d, bqk_d, wv_d, bv_d, wo_d, out_d):
    with (
        tc.tile_pool(name=f"persist{p}", bufs=1) as pers,
        tc.tile_pool(name=f"wpool{p}", bufs=1) as wpool,
    ):
        # persistent per-pass tensors
        qkT = [pers.tile([128, T], MMDT, name=f"qkT{p}_{m}", tag=f"qkT{m}")
               for m in range(4)]                       # m 0,1 = q; 2,3 = k
        vA = [pers.tile([128, NH * 65], MMDT, name=f"vA{p}_{i}", tag=f"vA{i}")
              for i in range(NKT)]                      # [tok, (h, hd+1)]
        yT2 = [pers.tile([128, T], MMDT, name=f"yT2{p}_{hp}", tag=f"yT2{hp}")
               for hp in range(NH // 2)]
        dgather = pers.tile([16, 512], f32, name=f"dg{p}", tag="dg")
        rgather = pers.tile([16, 512], f32, name=f"rg{p}", tag="rg")

        # weights
        wqk_sb = [wpool.tile([128, 2 * F], MMDT, name=f"wqk{p}_{k}", tag=f"wqk{k}")
                  for k in range(8)]
        wv_sb = [wpool.tile([128, NH * 65], MMDT, name=f"wv{p}_{k}", tag=f"wv{k}")
                 for k in range(8)]
        bqk_sb = [wpool.tile([128, 1], f32, name=f"bqk{p}_{m}", tag=f"bqk{m}")
                  for m in range(4)]
        bv_sb = wpool.tile([1, NH * 65], MMDT, name=f"bv{p}", tag="bv")
        wo_sb = [wpool.tile([128, D], MMDT, name=f"wo{p}_{hp}", tag=f"wo{hp}")
                 for hp in range(NH // 2)]
        for k in range(8):
            nc.sync.dma_start(wqk_sb[k][:], wqk_d[k * 128:(k + 1) * 128, p, :])
            nc.sync.dma_start(wv_sb[k][:], wv_d[k * 128:(k + 1) * 128, p, :])
        for m in range(4):
            nc.sync.dma_start(bqk_sb[m][:], bqk_d[p, m * 128:(m + 1) * 128, :])
        nc.sync.dma_start(bv_sb[:], bv_d[p])
        for hp in range(NH // 2):
            nc.sync.dma_start(wo_sb[hp][:], wo_d[p, hp])

        # ---- projection (consumes shared xT) ----
        with (
            tc.tile_pool(name=f"pj_ps{p}", bufs=2, space="PSUM") as pj_ps,
        ):
            for ch in range(NCH):
                c0_, c1_ = ch * CH, (ch + 1) * CH
                # q,k projection: feature-major
                for m in range(4):
                    ps = pj_ps.tile([128, CH], f32, name="qk_ps", tag="qk_ps")
                    for k in range(8):
                        nc.tensor.matmul(
                            ps[:], wqk_sb[k][:, m * 128:(m + 1) * 128],
                            xT[k][:, c0_:c1_], start=(k == 0), stop=(k == 7))
                    nc.vector.tensor_scalar_add(
                        qkT[m][:, c0_:c1_], ps[:], bqk_sb[m][:])
                # v projection: token-major, with bias via K=1 matmul
                for tt in range(4):
                    ps = pj_ps.tile([128, NH * 65], f32, name="v_ps", tag="v_ps")
                    for k in range(8):
                        nc.tensor.matmul(
                            ps[:], xT[k][:, ch * CH + tt * 128:
                                         ch * CH + (tt + 1) * 128], wv_sb[k][:],
                            start=(k == 0), stop=False)
                    nc.tensor.matmul(ps[:], ones_tok[:], bv_sb[:],
                                     start=False, stop=True)
                    ti = ch * 4 + tt
                    nc.vector.tensor_copy(vA[ti][:], ps[:])

        # ---- attention ----
        with (
            tc.tile_pool(name=f"sc_ps{p}", bufs=3, space="PSUM") as sc_ps,
            tc.tile_pool(name=f"pv_ps{p}", bufs=2, space="PSUM") as pv_ps,
            tc.tile_pool(name=f"bc_ps{p}", bufs=2, space="PSUM") as bc_ps,
            tc.tile_pool(name=f"att_sb{p}", bufs=4) as att_sb,
            tc.tile_pool(name=f"dt_sb{p}", bufs=4) as dt_sb,
        ):
            for hp in range(NH // 2):            # head pairs share row groups
                for qj in range(NQJ):
                    kis = [ki for ki in range(NKT) if blocks[(ki, qj)] != "skip"]
                    qt, kt = qkT[hp], qkT[2 + hp]
                    q0 = qj * 512
                    pvs = [pv_ps.tile([65, 512], f32, name="pv", tag="pv")
                           for _ in range(2)]
                    for i, ki in enumerate(kis):
                        blk = blocks[(ki, qj)]
                        if blk == "full":
                            o, w0, w1, u = 0, 0, 0, None
                        else:
                            u, o, w0, w1 = blk
                        n = 512 - o
                        pts = []
                        for hh in range(2):
                            r0 = hh * 64
                            sc = sc_ps.tile([128, 512], f32, name="sc", tag="sc")
                            nc.tensor.matmul(
                                sc[:, o:512],
                                kt[r0:r0 + 64, ki * 128:(ki + 1) * 128],
                                qt[r0:r0 + 64, q0 + o:q0 + 512],
                                start=True, stop=True)
                            if u is not None and w1 > w0:
                                nc.vector.tensor_add(
                                    sc[:, w0:w1], sc[:, w0:w1],
                                    pm_sb[u][:, w0:w1])
                            pt = att_sb.tile([128, 512], MMDT, name="pt", tag="pt")
                            nc.scalar.activation(pt[:, o:512], sc[:, o:512],
                                                 AF.Exp, scale=0.125)
                            pts.append(pt)
                        for hh in range(2):
                            h = hp * 2 + hh
                            va = vA[ki].rearrange("p (h w) -> p h w", w=65)
                            nc.tensor.matmul(
                                pvs[hh][:, o:512], va[:, h, :], pts[hh][:, o:512],
                                start=(i == 0), stop=(i == len(kis) - 1))
                    for hh in range(2):
                        h = hp * 2 + hh
                        r0 = hh * 64
                        pv = pvs[hh]
                        nc.scalar.copy(
                            yT2[hp][r0:r0 + 64, qj * 512:(qj + 1) * 512],
                            pv[0:64, :])
                        r = h * NQJ + qj
                        dtmp = dt_sb.tile([1, 512], f32, name="dtmp", tag="dtmp")
                        nc.vector.tensor_copy(dtmp[:], pv[64:65, :])
                        nc.sync.dma_start(dgather[r:r + 1, :], dtmp[:])
            # batched reciprocal + per-pair broadcast + in-place normalize
            nc.vector.reciprocal(rgather[:], dgather[:])
            for hp in range(NH // 2):
                for qj in range(NQJ):
                    bc = bc_ps.tile([128, 512], f32, name="bc", tag="bc")
                    nc.tensor.matmul(bc[:], sel_sb[hp * NQJ + qj][:], rgather[:],
                                     start=True, stop=True)
                    ysl = yT2[hp][:, qj * 512:(qj + 1) * 512]
                    nc.vector.tensor_mul(ysl, ysl, bc[:])

        # ---- out-projection ----
        with (
            tc.tile_pool(name=f"o_ps{p}", bufs=2, space="PSUM") as o_ps,
            tc.tile_pool(name=f"o_sb{p}", bufs=4) as o_sb,
        ):
            for tt in range(NKT):
                for n in range(2):
                    ps = o_ps.tile([128, 512], f32, name="o_ps", tag="o_ps")
                    for hp in range(NH // 2):
                        nc.tensor.matmul(
                            ps[:],
                            yT2[hp][:, tt * 128:(tt + 1) * 128],
                            wo_sb[hp][:, n * 512:(n + 1) * 512],
                            start=(hp == 0), stop=(hp == NH // 2 - 1))
                    ob = o_sb.tile([128, 512], f32, name="ob", tag="ob")
                    nc.vector.tensor_copy(ob[:], ps[:])
                    nc.sync.dma_start(
                        out_d[p, tt * 128:(tt + 1) * 128, n * 512:(n + 1) * 512],
                        ob[:])


def kernel(x, mask, w_qkv, b_qkv, w_out, b_out):
    global LAST_RESULTS
    x = np.ascontiguousarray(np.asarray(x, np.float32))
    mask2d = np.asarray(mask, bool).reshape(T, T)
    w_qkv = np.asarray(w_qkv, np.float32)
    b_qkv = np.asarray(b_qkv, np.float32)
    w_out = np.asarray(w_out, np.float32)
    b_out = np.asarray(b_out, np.float32)

    blocks, patterns = _classify_blocks(mask2d)
    key = (MMDT, patterns.tobytes(), tuple(sorted(blocks.items())).__hash__())
    if key not in _CACHE:
        _CACHE[key] = _build(blocks, len(patterns))
    nc = _CACHE[key]

    ident = np.eye(128, dtype=np.float32)
    sel = np.zeros((8, 16, 128), np.float32)
    for hp in range(2):
        for qj in range(4):
            sel[hp * 4 + qj, (2 * hp) * 4 + qj, 0:64] = 1.0
            sel[hp * 4 + qj, (2 * hp + 1) * 4 + qj, 64:128] = 1.0

    in_maps = []
    for c in range(8):
        b, hg = c // 2, c % 2
        # global head range for this core: hg*8 .. hg*8+8, in 2 passes of 4
        wqk = np.empty((D, NPASS, 2 * F), np.float32)
        bqk = np.empty((NPASS, 2 * F, 1), np.float32)
        wv = np.zeros((D, NPASS, NH * 65), np.float32)
        bv = np.zeros((NPASS, 1, NH * 65), np.float32)
        wo = np.empty((NPASS, NH // 2, 128, D), np.float32)
        for p in range(NPASS):
            h0 = hg * 8 + p * NH          # first global head of this pass
            c0 = h0 * HD                  # feature offset
            wqk[:, p, 0:F] = w_qkv[:, c0:c0 + F]
            wqk[:, p, F:2 * F] = w_qkv[:, D + c0:D + c0 + F]
            bqk[p, 0:F, 0] = b_qkv[c0:c0 + F]
            bqk[p, F:2 * F, 0] = b_qkv[D + c0:D + c0 + F]
            for h in range(NH):
                cs = 2 * D + c0 + h * HD
                wv[:, p, h * 65:h * 65 + 64] = w_qkv[:, cs:cs + HD]
                bv[p, 0, h * 65:h * 65 + 64] = b_qkv[cs:cs + HD]
                bv[p, 0, h * 65 + 64] = 1.0
            for hp in range(NH // 2):
                wo[p, hp] = w_out[c0 + hp * 128:c0 + (hp + 1) * 128, :]
        in_maps.append({
            "xb": np.ascontiguousarray(x[b]),
            "wqk": wqk, "bqk": bqk, "wv": wv, "bv": bv, "wo": wo,
            "pm": patterns, "ident": ident, "sel": sel,
            "onesd": np.ones((1, 128), np.float32),
        })

    trace = os.environ.get("KERNEL_TRACE") == "1"
    LAST_RESULTS = run_bass_kernel_spmd(
        nc, in_maps, list(range(8)), trace=trace)
    res = LAST_RESULTS.results

    out = np.empty((B, T, D), np.float32)
    for b in range(B):
        acc = res[2 * b]["out"][0] + res[2 * b]["out"][1] \
            + res[2 * b + 1]["out"][0] + res[2 * b + 1]["out"][1]
        out[b] = acc + b_out
    return out
